# revision 27
# baseline (speedup 1.0000x reference)
"""Trainium2 Bass kernel for nn_LocalAttention (depthwise causal conv + RoPE +
windowed local attention), data-parallel over the batch dim on 8 NeuronCores.

Self-contained: hardcodes shapes B=32, N=4096, D=64, WS=128 and the sharding
(4 batches per core). Host-side prep is limited to dtype casts and layout
transforms (Toeplitz band matrices from the depthwise conv weights, RoPE
cos/sin tables, contiguous-DMA reshapes); all FLOPs over the activations run
on device.

v2 layout/schedule notes:
  - all DMA patterns are contiguous per partition (>=8KB runs) and split
    across both HWDGE queues (sync + activation engine)
  - conv output layout (d, b, w) so PSUM drains are flat copies
  - q/k transposes to head-dim-major layout are plain matmuls against an
    identity moving operand (much faster than PE transpose mode)
  - sim matmuls for a batch pair run as two concurrent 64x128 row tiles
  - softmax denominator comes from a ones-column appended to V
  - rope and attention are chunked per batch-pair so all engines pipeline
"""

import sys

sys.path.insert(0, "/opt/trn_rl_repo")

import ml_dtypes
import numpy as np

import concourse.bass as bass
import concourse.mybir as mybir
import concourse.tile as tile
from concourse.bass_utils import run_bass_kernel_spmd
from concourse.masks import make_identity

BF16 = mybir.dt.bfloat16
F32 = mybir.dt.float32
NPBF = ml_dtypes.bfloat16

B, N, D, WS = 32, 4096, 64, 128
W = N // WS              # 32 windows
NCORES = 8
BL = B // NCORES         # 4 batches per core
NWP = W + 1              # 33 window slots (slot 0 = zero pad = "window -1")
SCALE = D ** -0.5
ROPE_BASE = 10000.0

XCOLS = D * W * BL       # xt cols: (d, w, b) = 8192
CCOLS = D * W * BL       # qc/kc cols: (d, w, b) = 8192
VCOLS = (D + 1) * W * BL  # vc cols: (d(+ones), w, b) = 8320
PCOLS = 2 * W * 2 * WS   # p cols per pair: (b2, m, 256) = 16384
TD_CH = 4                # toeplitz chunk: channels per DMA
TCH_COLS = TD_CH * 2 * WS  # 1024
XD_CH = 8                # x chunk: channels per DMA
XCH_COLS = XD_CH * W * BL  # 2048


def _split_multiwaits(nc, max_waits=1):
    """walrus in this env rejects >1 sem wait per instruction; split extras
    into standalone NoOp waits inserted just before, on the same engine."""
    n_fixed = 0
    for fn in nc.m.functions:
        for bb in fn.blocks:
            insts = bb.instructions
            new_list = []
            changed = False
            for inst in insts:
                si = inst.sync_info
                if si is not None and si.on_wait and len(si.on_wait) > max_waits:
                    waits = list(si.on_wait)
                    for w in waits[:-max_waits]:
                        nop = mybir.InstNoOp(
                            name=f"{inst.name}-xw{n_fixed}",
                            engine=inst.engine,
                            ins=[],
                            outs=[],
                            sync_info=mybir.SyncInfo(on_wait=[w], on_update=[]),
                        )
                        new_list.append(nop)
                        n_fixed += 1
                    si.on_wait = waits[-max_waits:]
                    changed = True
                new_list.append(inst)
            if changed:
                bb.instructions = new_list
    return n_fixed


def _ap(t, offset, dims):
    """AP over tile/dram tensor t: partition dim kept, free dims replaced."""
    return bass.AP(tensor=t.tensor, offset=t.offset + offset, ap=[t.ap[0]] + dims)


def _build_program():
    nc = bass.Bass()
    # x: [128 j, (d, w, b)] host-pretransposed, fully contiguous
    xq = nc.dram_tensor("xq", [128, W * BL * D], BF16, kind="ExternalInput")
    xk = nc.dram_tensor("xk", [128, W * BL * D], BF16, kind="ExternalInput")
    xv = nc.dram_tensor("xv", [128, W * BL * D], BF16, kind="ExternalInput")
    # toeplitz: [128 j, (d, half, i)] contiguous
    tq = nc.dram_tensor("tq", [128, D * 2 * WS], BF16, kind="ExternalInput")
    tk = nc.dram_tensor("tk", [128, D * 2 * WS], BF16, kind="ExternalInput")
    tv = nc.dram_tensor("tv", [128, D * 2 * WS], BF16, kind="ExternalInput")
    # rope tables: [128 i, (d, w, b2)] (replicated over the 2 pair batches)
    cosb = nc.dram_tensor("cosb", [128, D * W * 2], BF16, kind="ExternalInput")
    sinb = nc.dram_tensor("sinb", [128, D * W * 2], BF16, kind="ExternalInput")
    out = nc.dram_tensor("out", [BL, N, D], F32, kind="ExternalOutput")

    xdram = {"q": xq, "k": xk, "v": xv}
    tdram = {"q": tq, "k": tk, "v": tv}

    with tile.TileContext(nc) as tc:
        import contextlib

        with contextlib.ExitStack() as ctx:
            const = ctx.enter_context(tc.tile_pool(name="const", bufs=1))
            xpool = ctx.enter_context(tc.tile_pool(name="x", bufs=2))
            tpool = ctx.enter_context(tc.tile_pool(name="toep", bufs=3))
            tabs = ctx.enter_context(tc.tile_pool(name="tabs", bufs=1))
            big = ctx.enter_context(tc.tile_pool(name="big", bufs=2))
            rtmp = ctx.enter_context(tc.tile_pool(name="rtmp", bufs=2))
            vpool = ctx.enter_context(tc.tile_pool(name="v", bufs=1))
            qtp = ctx.enter_context(tc.tile_pool(name="qt", bufs=4))
            ppool = ctx.enter_context(tc.tile_pool(name="p", bufs=1))
            opool = ctx.enter_context(tc.tile_pool(name="o", bufs=2))
            spool = ctx.enter_context(tc.tile_pool(name="s", bufs=2))

            # ---- input DMAs, earliest-needed first, split across queues.
            # sync queue: x tensors + rope tables; act queue: toeplitz chunks.
            xt = {}
            for name in ("q", "k", "v"):
                t = xpool.tile([128, XCOLS], BF16, tag="x")
                xt[name] = t
            tchunks = {}
            for name in ("q", "k", "v"):
                t = xt[name]
                for c in range(D // TD_CH):
                    tt = tpool.tile([128, TCH_COLS], BF16, tag="toep")
                    src = _ap_dram(tdram[name], c * TCH_COLS, [[1, TCH_COLS]],
                                   D * 2 * WS)
                    nc.scalar.dma_start(out=tt[:], in_=src)
                    tchunks[(name, c)] = tt
                for c in range(D // XD_CH):
                    src = _ap_dram(xdram[name], c * XCH_COLS, [[1, XCH_COLS]],
                                   W * BL * D)
                    nc.sync.dma_start(
                        out=t[:, c * XCH_COLS: (c + 1) * XCH_COLS], in_=src)

            costab = tabs.tile([128, D * W * 2], BF16)
            nc.sync.dma_start(
                out=costab[:], in_=_ap_dram(cosb, 0, [[1, D * W * 2]], D * W * 2))
            sintab = tabs.tile([128, D * W * 2], BF16)
            nc.sync.dma_start(
                out=sintab[:], in_=_ap_dram(sinb, 0, [[1, D * W * 2]], D * W * 2))

            # constants
            ident = const.tile([128, 128], BF16)
            make_identity(nc, ident)
            tri = const.tile([128, 128], BF16)  # tri[j,i] = 1 if i>=j else 0
            nc.vector.memset(tri[:], 1.0)
            nc.gpsimd.affine_select(
                out=tri[:], in_=tri[:], compare_op=mybir.AluOpType.is_ge,
                fill=0.0, base=0, channel_multiplier=-1, pattern=[[1, 128]],
            )

            # persistent activation tiles
            qc = big.tile([128, CCOLS], BF16, tag="big")   # (d, b, w)
            kc = big.tile([128, CCOLS], BF16, tag="big")
            vc = vpool.tile([128, VCOLS], BF16)            # (d|ones, b, w)
            nc.vector.memset(vc[:, D * BL * W:], 1.0)      # ones column block

            qT = {}  # (tensor, pair) -> [128 (b2,d), (w,i)] bf16

            with tc.tile_pool(name="convps", bufs=6, space="PSUM") as convps, \
                 tc.tile_pool(name="tps", bufs=2, space="PSUM") as tps:
                # ---- conv via per-channel Toeplitz matmuls
                # cp psum cols: (dd 8, w 32, b 4); drains are flat copies
                # into (d, w, b)-layout activation tiles.
                drain_alt = 0
                for name in ("q", "k", "v"):
                    x = xt[name]
                    dstt = {"q": qc, "k": kc, "v": vc}[name]
                    for g in range(16):  # 4 channels per psum group
                        cp = convps.tile([128, 4 * 128], F32)
                        for dd in range(4):
                            d = g * 4 + dd
                            tt = tchunks[(name, d // TD_CH)]
                            dl = d % TD_CH
                            lo = tt[:, (dl * 2) * 128: (dl * 2 + 1) * 128]
                            hi = tt[:, (dl * 2 + 1) * 128: (dl * 2 + 2) * 128]
                            # x cols (d, w, b): own windows = flat 128 cols;
                            # prev windows (w>=1 only; w=0's prev is the
                            # zero pad, so its psum region gets hi only)
                            rhs_hi = _ap(x, d * W * BL, [[1, W * BL]])
                            rhs_lo = _ap(x, d * W * BL, [[1, (W - 1) * BL]])
                            ps = cp[:, dd * 128: (dd + 1) * 128]
                            ps_lo = cp[:, dd * 128 + BL: (dd + 1) * 128]
                            nc.tensor.matmul(ps, hi, rhs_hi, start=True, stop=False)
                            nc.tensor.matmul(ps_lo, lo, rhs_lo, start=False,
                                             stop=True, skip_group_check=True)
                        # flat drain: cp (dd, w, b) -> dstt cols (d, w, b)
                        dst = dstt[:, g * 512: (g + 1) * 512]
                        if drain_alt % 3 == 2:
                            nc.scalar.copy(dst, cp[:])
                        else:
                            nc.vector.tensor_copy(dst, cp[:])
                        drain_alt += 1

                    # ---- RoPE per batch pair right after this tensor's conv
                    # (v has no rope). q on DVE, k on GpSimd. 2-op in-place:
                    # t2 = partner(x)*sin; x *= cos; x += t2
                    if name != "v":
                        for pair in range(2):
                            eng = nc.vector
                            po = pair * 2  # col offset of batch pair
                            t2 = rtmp.tile([128, D * W * 2], BF16, tag="rt")
                            # dims (d, w, b2)
                            xv_ = _ap(dstt, po, [[W * BL, D], [BL, W], [1, 2]])
                            cos_in = _ap(costab, 0,
                                         [[W * 2, D], [2, W], [1, 2]])
                            # partner products, split by even/odd output d:
                            # t2[2u] = x[2u+1]*sin[2u]; t2[2u+1] = x[2u]*sin[2u+1]
                            for e in range(2):
                                part_in = _ap(
                                    dstt, po + (1 - e) * W * BL,
                                    [[2 * W * BL, D // 2], [BL, W], [1, 2]])
                                sin_in = _ap(
                                    sintab, e * W * 2,
                                    [[4 * W, D // 2], [2, W], [1, 2]])
                                t2v = _ap(
                                    t2, e * W * 2,
                                    [[4 * W, D // 2], [2, W], [1, 2]])
                                eng.tensor_mul(t2v, part_in, sin_in)
                            eng.tensor_mul(xv_, xv_, cos_in)
                            sh = [[W * 2, D], [2, W], [1, 2]]
                            eng.tensor_add(xv_, xv_, _ap(t2, 0, sh))

                # ---- transposes pair 0 (pair 1 happens inside attention
                # phase, between the two pairs' attention blocks)
                def emit_transposes(pair, pool):
                    for name in ("q", "k"):
                        srct = qc if name == "q" else kc
                        qt = qtp.tile([128, W * WS], BF16, tag="qt",
                                      name=f"qt_{name}{pair}")
                        for w4 in range(8):
                            tp = pool.tile([128, 512], F32, name=f"tp{pair}")
                            for wi in range(4):
                                w = w4 * 4 + wi
                                for b2 in range(2):
                                    # lhsT cols = d at (w, b); single stride
                                    src = _ap(srct,
                                              w * BL + pair * 2 + b2,
                                              [[W * BL, D]])
                                    nc.tensor.matmul(
                                        tp[b2 * 64: b2 * 64 + 64,
                                           wi * 128: (wi + 1) * 128],
                                        src, ident[:], start=True, stop=True,
                                    )
                            dst = qt[:, w4 * 512: (w4 + 1) * 512]
                            nc.scalar.copy(dst, tp[:])
                        qT[(name, pair)] = qt

                emit_transposes(0, tps)

            # ---- attention per batch pair
            with tc.tile_pool(name="simps", bufs=2, space="PSUM") as simps, \
                 tc.tile_pool(name="avps", bufs=2, space="PSUM") as avps, \
                 tc.tile_pool(name="tps2", bufs=2, space="PSUM") as tps2:
                PB = W * 2 * WS  # 8192: p cols per batch

                def emit_attention(pair):
                    qt = qT[("q", pair)]
                    kt = qT[("k", pair)]
                    p = ppool.tile([128, PCOLS], BF16, tag="p")
                    # sim passes, 2 m's per group, both batches via row tiles
                    for g2 in range(16):
                        sp = simps.tile([128, 1024], F32)
                        m0 = g2 * 2
                        ncol_g = 0
                        for mi in range(2):
                            m = m0 + mi
                            ncols = 256 if m < W - 1 else 128
                            for h in range(2):  # batch half (row tile)
                                nc.tensor.matmul(
                                    sp[:, h * 512 + mi * 256:
                                       h * 512 + mi * 256 + ncols],
                                    kt[h * 64: h * 64 + 64,
                                       m * 128: (m + 1) * 128],
                                    qt[h * 64: h * 64 + 64,
                                       m * 128: m * 128 + ncols],
                                    start=True, stop=True,
                                )
                            ncol_g += ncols
                        # exp of both banks -> p[(b2, m0..m0+1, :)]
                        esrc = _ap(sp, 0, [[512, 2], [1, ncol_g]])
                        edst = _ap(p, m0 * 256, [[PB, 2], [1, ncol_g]])
                        nc.scalar.activation(
                            edst, esrc, mybir.ActivationFunctionType.Exp)
                    # pad-row fixup (global key position 0 fully masked)
                    for b2 in range(2):
                        nc.vector.memset(p[0:1, b2 * PB: b2 * PB + 256], 0.0)
                    # causal mask: own-halves *= tri; per 8-m chunk on DVE
                    for mc in range(4):
                        pview = _ap(p, mc * 8 * 256,
                                    [[PB, 2], [256, 8], [1, 128]])
                        tri_b = _ap(tri, 0, [[0, 2], [0, 8], [1, 128]])
                        nc.vector.tensor_mul(pview, pview, tri_b)
                    # all-masked-row fixup: query 0 attends uniformly
                    for b2 in range(2):
                        nc.vector.memset(p[:, b2 * PB: b2 * PB + 1], 1.0)

                    # AV in 7-window chunks per batch (one PSUM bank each)
                    for b2 in range(2):
                        b = pair * 2 + b2
                        pb = b2 * PB
                        sr = spool.tile([128, 8], F32, tag="sr")
                        for chunk in range(5):
                            w0 = chunk * 7
                            nwin = min(7, W - w0)
                            ot = opool.tile([128, 7 * D], F32, tag="ot")
                            av = avps.tile([128, 512], F32)
                            for k in range(nwin):
                                w = w0 + k
                                own = p[:, pb + w * 256: pb + w * 256 + 128]
                                ov = av[:, k * 65: k * 65 + 65]
                                vw = _ap(vc, w * BL + b, [[W * BL, D + 1]])
                                first_only = w == 0
                                nc.tensor.matmul(ov, own, vw, start=True,
                                                 stop=first_only)
                                if w > 0:
                                    prev = p[:, pb + (w - 1) * 256 + 128:
                                             pb + w * 256]
                                    vprev = _ap(vc, (w - 1) * BL + b,
                                                [[W * BL, D + 1]])
                                    nc.tensor.matmul(ov, prev, vprev,
                                                     start=False, stop=True,
                                                     skip_group_check=True)
                            if chunk == 0:
                                # window-0 query-0 sum correction (+128 pad)
                                nc.vector.tensor_scalar_add(
                                    av[0:1, 64:65], av[0:1, 64:65], 128.0)
                            # normalize: recip of s, broadcast-mul
                            nc.vector.reciprocal(
                                sr[:, :nwin], _ap(av, 64, [[65, nwin]]))
                            avv = _ap(av, 0, [[65, nwin], [1, D]])
                            srv = _ap(sr, 0, [[1, nwin], [0, D]])
                            otv = _ap(ot, 0, [[D, nwin], [1, D]])
                            nc.vector.tensor_mul(otv, avv, srv)
                            dstd = bass.AP(
                                tensor=out, offset=b * N * D + w0 * WS * D,
                                ap=[[D, 128], [WS * D, nwin], [1, D]],
                            )
                            nc.sync.dma_start(
                                out=dstd, in_=_ap(ot, 0, [[D, nwin], [1, D]]))

                emit_attention(0)
                emit_transposes(1, tps2)
                emit_attention(1)

    _split_multiwaits(nc)
    return nc


def _ap_dram(t, offset, dims, row_elems):
    """AP over a [128, row_elems] dram tensor: partition stride row_elems."""
    return bass.AP(tensor=t, offset=offset, ap=[[row_elems, 128]] + dims)


_PROG = None


def _get_prog():
    global _PROG
    if _PROG is None:
        _PROG = _build_program()
    return _PROG


def _host_prep(q, k, v, wq, wk, wv):
    """Build per-core input maps (bf16 casts + layout transforms)."""
    # Toeplitz bands: lo[j,i] = w[d, j-i-1] (prev window), hi[j,i] =
    # w[d, j-i+127] (own window); DRAM layout [j, (d, half, i)] contiguous.
    jj = np.arange(WS)[:, None]
    ii = np.arange(WS)[None, :]
    lod = jj - ii - 1
    hid = jj - ii + (WS - 1)
    lom = (lod >= 0) & (lod < WS)
    him = (hid >= 0) & (hid < WS)
    lodc = np.clip(lod, 0, WS - 1)
    hidc = np.clip(hid, 0, WS - 1)

    def toep(w, scale=1.0):
        wd = np.asarray(w, np.float32).reshape(D, WS) * scale
        t = np.zeros((D, 2, WS, WS), np.float32)
        t[:, 0][:, lom] = wd[:, lodc[lom]]
        t[:, 1][:, him] = wd[:, hidc[him]]
        # [d, half, j, i] -> [j, (d, half, i)]
        return np.ascontiguousarray(t.transpose(2, 0, 1, 3)).reshape(
            WS, D * 2 * WS).astype(NPBF)

    tq_np = toep(wq, SCALE)
    tk_np = toep(wk)
    tv_np = toep(wv)

    theta = 1.0 / ROPE_BASE ** (np.arange(0, D, 2, dtype=np.float32) / D)
    pm = np.arange(N, dtype=np.float32)[:, None] * theta[None, :]
    cos = np.repeat(np.cos(pm), 2, axis=-1)  # [n, d]
    sin = np.repeat(np.sin(pm), 2, axis=-1)
    sgn = np.where(np.arange(D) % 2 == 0, -1.0, 1.0).astype(np.float32)
    # [i, (d, w, b2)] layout, replicated over the 2 batches of a pair
    def rope_table(t):
        tt = np.ascontiguousarray(t.reshape(W, WS, D).transpose(1, 2, 0))
        tt = np.repeat(tt.reshape(WS, D * W, 1), 2, axis=2)
        return np.ascontiguousarray(tt).reshape(WS, D * W * 2).astype(NPBF)

    cosb_np = rope_table(cos)
    sinb_np = rope_table(sin * sgn[None, :])

    def xprep(x, sl):
        # [BL, N, D] -> [j, (d, w, b)] contiguous bf16
        xb = np.asarray(x[sl], np.float32).reshape(BL, W, WS, D)
        return np.ascontiguousarray(xb.transpose(2, 3, 1, 0)).reshape(
            WS, D * W * BL).astype(NPBF)

    in_maps = []
    for c in range(NCORES):
        sl = slice(c * BL, (c + 1) * BL)
        in_maps.append({
            "xq": xprep(q, sl),
            "xk": xprep(k, sl),
            "xv": xprep(v, sl),
            "tq": tq_np, "tk": tk_np, "tv": tv_np,
            "cosb": cosb_np, "sinb": sinb_np,
        })
    return in_maps


def _install_ntff_hook():
    """Provide antenv.axon_hooks with a ctypes NTFF profile hook (the slim
    container lacks it); enables trace=True under axon."""
    import sys as _sys
    import types
    import ctypes
    import contextlib

    try:
        from antenv.axon_hooks import get_axon_ntff_profile_hook  # noqa: F401
        return
    except ImportError:
        pass
    so_path = "/opt/axon/libaxon_pjrt.so"
    try:
        lib = ctypes.CDLL(so_path)
    except OSError:
        return
    if not hasattr(lib, "axon_start_nrt_profile"):
        return
    lib.axon_start_nrt_profile.argtypes = [
        ctypes.POINTER(ctypes.c_int64), ctypes.c_size_t]
    lib.axon_start_nrt_profile.restype = ctypes.c_int64
    lib.axon_stop_nrt_profile.argtypes = [ctypes.c_char_p]
    lib.axon_stop_nrt_profile.restype = ctypes.c_int64

    @contextlib.contextmanager
    def _hook(output_dir, device_ids):
        import jax
        jax.devices()
        if device_ids:
            ids = (ctypes.c_int64 * len(device_ids))(*device_ids)
            rc = lib.axon_start_nrt_profile(ids, len(device_ids))
        else:
            rc = lib.axon_start_nrt_profile(None, 0)
        if rc != 0:
            raise RuntimeError(f"axon_start_nrt_profile rc={rc}")
        try:
            yield
        finally:
            n = lib.axon_stop_nrt_profile(str(output_dir).encode())
            print(f"profile: {n} file(s) written to {output_dir}")

    import antenv

    mod = types.ModuleType("antenv.axon_hooks")
    _state = {"hook": _hook}
    mod.set_axon_ntff_profile_hook = lambda h: _state.__setitem__("hook", h)
    mod.get_axon_ntff_profile_hook = lambda: _state["hook"]
    _sys.modules["antenv.axon_hooks"] = mod
    antenv.axon_hooks = mod


def run(q, k, v, wq, wk, wv, trace=False):
    nc = _get_prog()
    in_maps = _host_prep(q, k, v, wq, wk, wv)
    if trace:
        _install_ntff_hook()
    res = run_bass_kernel_spmd(nc, in_maps, core_ids=list(range(NCORES)),
                               trace=trace)
    outp = np.concatenate([res.results[c]["out"] for c in range(NCORES)], axis=0)
    return outp, res


def kernel(q, k, v, wq, wk, wv):
    outp, _ = run(q, k, v, wq, wk, wv)
    return outp


# revision 29
# speedup vs baseline: 1.1395x; 1.1395x over previous
"""Trainium2 Bass kernel for nn_LocalAttention (depthwise causal conv + RoPE +
windowed local attention), data-parallel over the batch dim on 8 NeuronCores.

Self-contained: hardcodes shapes B=32, N=4096, D=64, WS=128 and the sharding
(4 batches per core). Host-side prep is limited to dtype casts and layout
transforms (Toeplitz band matrices from the depthwise conv weights, RoPE
cos/sin tables, contiguous-DMA reshapes); all FLOPs over the activations run
on device.

v2 layout/schedule notes:
  - all DMA patterns are contiguous per partition (>=8KB runs) and split
    across both HWDGE queues (sync + activation engine)
  - conv output layout (d, b, w) so PSUM drains are flat copies
  - q/k transposes to head-dim-major layout are plain matmuls against an
    identity moving operand (much faster than PE transpose mode)
  - sim matmuls for a batch pair run as two concurrent 64x128 row tiles
  - softmax denominator comes from a ones-column appended to V
  - rope and attention are chunked per batch-pair so all engines pipeline
"""

import sys

sys.path.insert(0, "/opt/trn_rl_repo")

import ml_dtypes
import numpy as np

import concourse.bass as bass
import concourse.mybir as mybir
import concourse.tile as tile
from concourse.bass_utils import run_bass_kernel_spmd
from concourse.masks import make_identity

BF16 = mybir.dt.bfloat16
F32 = mybir.dt.float32
NPBF = ml_dtypes.bfloat16

B, N, D, WS = 32, 4096, 64, 128
W = N // WS              # 32 windows
NCORES = 8
BL = B // NCORES         # 4 batches per core
NWP = W + 1              # 33 window slots (slot 0 = zero pad = "window -1")
SCALE = D ** -0.5
ROPE_BASE = 10000.0

XCOLS = D * W * BL       # xt cols: (d, w, b) = 8192
CCOLS = D * W * BL       # qc/kc cols: (d, w, b) = 8192
VCOLS = (D + 1) * W * BL  # vc cols: (d(+ones), w, b) = 8320
PCOLS = 2 * W * 2 * WS   # p cols per pair: (b2, m, 256) = 16384
TD_CH = 8                # toeplitz chunk: channels per DMA
TCH_COLS = TD_CH * 2 * WS  # 2048
XD_CH = 16               # x chunk: channels per DMA
XCH_COLS = XD_CH * W * BL  # 4096


def _split_multiwaits(nc, max_waits=1):
    """walrus in this env rejects >1 sem wait per instruction; split extras
    into standalone NoOp waits inserted just before, on the same engine."""
    n_fixed = 0
    for fn in nc.m.functions:
        for bb in fn.blocks:
            insts = bb.instructions
            new_list = []
            changed = False
            for inst in insts:
                si = inst.sync_info
                if si is not None and si.on_wait and len(si.on_wait) > max_waits:
                    waits = list(si.on_wait)
                    for w in waits[:-max_waits]:
                        nop = mybir.InstNoOp(
                            name=f"{inst.name}-xw{n_fixed}",
                            engine=inst.engine,
                            ins=[],
                            outs=[],
                            sync_info=mybir.SyncInfo(on_wait=[w], on_update=[]),
                        )
                        new_list.append(nop)
                        n_fixed += 1
                    si.on_wait = waits[-max_waits:]
                    changed = True
                new_list.append(inst)
            if changed:
                bb.instructions = new_list
    return n_fixed


def _ap(t, offset, dims):
    """AP over tile/dram tensor t: partition dim kept, free dims replaced."""
    return bass.AP(tensor=t.tensor, offset=t.offset + offset, ap=[t.ap[0]] + dims)


def _build_program():
    nc = bass.Bass()
    # x: [128 j, (d, w, b)] host-pretransposed, fully contiguous
    xq = nc.dram_tensor("xq", [128, W * BL * D], BF16, kind="ExternalInput")
    xk = nc.dram_tensor("xk", [128, W * BL * D], BF16, kind="ExternalInput")
    xv = nc.dram_tensor("xv", [128, W * BL * D], BF16, kind="ExternalInput")
    # toeplitz: [128 j, (d, half, i)] contiguous
    tq = nc.dram_tensor("tq", [128, D * 2 * WS], BF16, kind="ExternalInput")
    tk = nc.dram_tensor("tk", [128, D * 2 * WS], BF16, kind="ExternalInput")
    tv = nc.dram_tensor("tv", [128, D * 2 * WS], BF16, kind="ExternalInput")
    # rope tables: [128 i, (d, w, b2)] (replicated over the 2 pair batches)
    cosb = nc.dram_tensor("cosb", [128, D * W * 2], BF16, kind="ExternalInput")
    sinb = nc.dram_tensor("sinb", [128, D * W * 2], BF16, kind="ExternalInput")
    out = nc.dram_tensor("out", [BL, N, D], F32, kind="ExternalOutput")

    xdram = {"q": xq, "k": xk, "v": xv}
    tdram = {"q": tq, "k": tk, "v": tv}

    with tile.TileContext(nc) as tc:
        import contextlib

        with contextlib.ExitStack() as ctx:
            const = ctx.enter_context(tc.tile_pool(name="const", bufs=1))
            xpool = ctx.enter_context(tc.tile_pool(name="x", bufs=2))
            tpool = ctx.enter_context(tc.tile_pool(name="toep", bufs=3))
            tabs = ctx.enter_context(tc.tile_pool(name="tabs", bufs=1))
            big = ctx.enter_context(tc.tile_pool(name="big", bufs=2))
            rtmp = ctx.enter_context(tc.tile_pool(name="rtmp", bufs=2))
            vpool = ctx.enter_context(tc.tile_pool(name="v", bufs=1))
            qtp = ctx.enter_context(tc.tile_pool(name="qt", bufs=4))
            ppool = ctx.enter_context(tc.tile_pool(name="p", bufs=1))
            opool = ctx.enter_context(tc.tile_pool(name="o", bufs=2))
            spool = ctx.enter_context(tc.tile_pool(name="s", bufs=2))

            # ---- input DMAs, earliest-needed first, split across queues.
            # sync queue: x tensors + rope tables; act queue: toeplitz chunks.
            xt = {}
            for name in ("q", "k", "v"):
                t = xpool.tile([128, XCOLS], BF16, tag="x")
                xt[name] = t
            tchunks = {}
            for name in ("q", "k", "v"):
                t = xt[name]
                for c in range(D // TD_CH):
                    tt = tpool.tile([128, TCH_COLS], BF16, tag="toep")
                    src = _ap_dram(tdram[name], c * TCH_COLS, [[1, TCH_COLS]],
                                   D * 2 * WS)
                    eng = nc.sync if (name == "q" and c == 0) else nc.scalar
                    eng.dma_start(out=tt[:], in_=src)
                    tchunks[(name, c)] = tt
                for c in range(D // XD_CH):
                    src = _ap_dram(xdram[name], c * XCH_COLS, [[1, XCH_COLS]],
                                   W * BL * D)
                    nc.sync.dma_start(
                        out=t[:, c * XCH_COLS: (c + 1) * XCH_COLS], in_=src)

            costab = tabs.tile([128, D * W * 2], BF16)
            nc.sync.dma_start(
                out=costab[:], in_=_ap_dram(cosb, 0, [[1, D * W * 2]], D * W * 2))
            sintab = tabs.tile([128, D * W * 2], BF16)
            nc.sync.dma_start(
                out=sintab[:], in_=_ap_dram(sinb, 0, [[1, D * W * 2]], D * W * 2))

            # constants
            ident = const.tile([128, 128], BF16)
            make_identity(nc, ident)
            tri = const.tile([128, 128], BF16)  # tri[j,i] = 1 if i>=j else 0
            nc.vector.memset(tri[:], 1.0)
            nc.gpsimd.affine_select(
                out=tri[:], in_=tri[:], compare_op=mybir.AluOpType.is_ge,
                fill=0.0, base=0, channel_multiplier=-1, pattern=[[1, 128]],
            )

            # persistent activation tiles
            qc = big.tile([128, CCOLS], BF16, tag="big")   # (d, b, w)
            kc = big.tile([128, CCOLS], BF16, tag="big")
            vc = vpool.tile([128, VCOLS], BF16)            # (d|ones, b, w)
            nc.vector.memset(vc[:, D * BL * W:], 1.0)      # ones column block

            qT = {}  # (tensor, pair) -> [128 (b2,d), (w,i)] bf16

            with tc.tile_pool(name="convps", bufs=3, space="PSUM") as convps, \
                 tc.tile_pool(name="tps", bufs=2, space="PSUM") as tps:
                # ---- conv via per-channel Toeplitz matmuls
                # cp psum cols: (dd 8, w 32, b 4); drains are flat copies
                # into (d, w, b)-layout activation tiles.
                drain_alt = 0
                for name in ("q", "k", "v"):
                    x = xt[name]
                    dstt = {"q": qc, "k": kc, "v": vc}[name]
                    for g in range(8):  # 8 channels per psum group
                        cp = convps.tile([128, 8 * 128], F32)
                        for dd in range(8):
                            d = g * 8 + dd
                            tt = tchunks[(name, d // TD_CH)]
                            dl = d % TD_CH
                            lo = tt[:, (dl * 2) * 128: (dl * 2 + 1) * 128]
                            hi = tt[:, (dl * 2 + 1) * 128: (dl * 2 + 2) * 128]
                            # x cols (d, w, b): own windows = flat 128 cols;
                            # prev windows (w>=1 only; w=0's prev is the
                            # zero pad, so its psum region gets hi only)
                            rhs_hi = _ap(x, d * W * BL, [[1, W * BL]])
                            rhs_lo = _ap(x, d * W * BL, [[1, (W - 1) * BL]])
                            ps = cp[:, dd * 128: (dd + 1) * 128]
                            ps_lo = cp[:, dd * 128 + BL: (dd + 1) * 128]
                            nc.tensor.matmul(ps, hi, rhs_hi, start=True, stop=False)
                            nc.tensor.matmul(ps_lo, lo, rhs_lo, start=False,
                                             stop=True, skip_group_check=True)
                        # flat drain: cp (dd, w, b) -> dstt cols (d, w, b)
                        dst = dstt[:, g * 1024: (g + 1) * 1024]
                        if drain_alt % 3 == 2:
                            nc.scalar.copy(dst, cp[:])
                        else:
                            nc.vector.tensor_copy(dst, cp[:])
                        drain_alt += 1

                    # ---- RoPE per batch pair right after this tensor's conv
                    # (v has no rope). q on DVE, k on GpSimd. 2-op in-place:
                    # t2 = partner(x)*sin; x *= cos; x += t2
                    if name != "v":
                        for pair in range(2):
                            eng = nc.vector
                            po = pair * 2  # col offset of batch pair
                            t2 = rtmp.tile([128, D * W * 2], BF16, tag="rt")
                            # dims (d, w, b2)
                            xv_ = _ap(dstt, po, [[W * BL, D], [BL, W], [1, 2]])
                            cos_in = _ap(costab, 0,
                                         [[W * 2, D], [2, W], [1, 2]])
                            # partner products, split by even/odd output d:
                            # t2[2u] = x[2u+1]*sin[2u]; t2[2u+1] = x[2u]*sin[2u+1]
                            for e in range(2):
                                part_in = _ap(
                                    dstt, po + (1 - e) * W * BL,
                                    [[2 * W * BL, D // 2], [BL, W], [1, 2]])
                                sin_in = _ap(
                                    sintab, e * W * 2,
                                    [[4 * W, D // 2], [2, W], [1, 2]])
                                t2v = _ap(
                                    t2, e * W * 2,
                                    [[4 * W, D // 2], [2, W], [1, 2]])
                                eng.tensor_mul(t2v, part_in, sin_in)
                            eng.tensor_mul(xv_, xv_, cos_in)
                            sh = [[W * 2, D], [2, W], [1, 2]]
                            eng.tensor_add(xv_, xv_, _ap(t2, 0, sh))

                # ---- transposes pair 0 (pair 1 happens inside attention
                # phase, between the two pairs' attention blocks)
                def emit_transposes(pair, pool):
                    for name in ("q", "k"):
                        srct = qc if name == "q" else kc
                        qt = qtp.tile([128, W * WS], BF16, tag="qt",
                                      name=f"qt_{name}{pair}")
                        for w4 in range(8):
                            tp = pool.tile([128, 512], F32, name=f"tp{pair}")
                            for wi in range(4):
                                w = w4 * 4 + wi
                                for b2 in range(2):
                                    # lhsT cols = d at (w, b); single stride
                                    src = _ap(srct,
                                              w * BL + pair * 2 + b2,
                                              [[W * BL, D]])
                                    nc.tensor.matmul(
                                        tp[b2 * 64: b2 * 64 + 64,
                                           wi * 128: (wi + 1) * 128],
                                        src, ident[:], start=True, stop=True,
                                    )
                            dst = qt[:, w4 * 512: (w4 + 1) * 512]
                            if pair == 0:
                                nc.scalar.copy(dst, tp[:])
                            else:
                                nc.vector.tensor_copy(dst, tp[:])
                        qT[(name, pair)] = qt

                emit_transposes(0, tps)

            # ---- attention per batch pair
            with tc.tile_pool(name="simps", bufs=2, space="PSUM") as simps, \
                 tc.tile_pool(name="avps", bufs=2, space="PSUM") as avps, \
                 tc.tile_pool(name="tps2", bufs=2, space="PSUM") as tps2:
                PB = W * 2 * WS  # 8192: p cols per batch

                def emit_attention(pair):
                    qt = qT[("q", pair)]
                    kt = qT[("k", pair)]
                    p = ppool.tile([128, PCOLS], BF16, tag="p")
                    # sim passes, 2 m's per group, both batches via row tiles
                    for g2 in range(16):
                        sp = simps.tile([128, 1024], F32)
                        m0 = g2 * 2
                        ncol_g = 0
                        for mi in range(2):
                            m = m0 + mi
                            ncols = 256 if m < W - 1 else 128
                            for h in range(2):  # batch half (row tile)
                                nc.tensor.matmul(
                                    sp[:, h * 512 + mi * 256:
                                       h * 512 + mi * 256 + ncols],
                                    kt[h * 64: h * 64 + 64,
                                       m * 128: (m + 1) * 128],
                                    qt[h * 64: h * 64 + 64,
                                       m * 128: m * 128 + ncols],
                                    start=True, stop=True,
                                )
                            ncol_g += ncols
                        # exp of both banks -> p[(b2, m0..m0+1, :)]
                        esrc = _ap(sp, 0, [[512, 2], [1, ncol_g]])
                        edst = _ap(p, m0 * 256, [[PB, 2], [1, ncol_g]])
                        nc.scalar.activation(
                            edst, esrc, mybir.ActivationFunctionType.Exp)
                    # pad-row fixup (global key position 0 fully masked)
                    for b2 in range(2):
                        nc.vector.memset(p[0:1, b2 * PB: b2 * PB + 256], 0.0)
                    # causal mask: own-halves *= tri; per 8-m chunk on DVE
                    for mc in range(4):
                        pview = _ap(p, mc * 8 * 256,
                                    [[PB, 2], [256, 8], [1, 128]])
                        tri_b = _ap(tri, 0, [[0, 2], [0, 8], [1, 128]])
                        nc.vector.tensor_mul(pview, pview, tri_b)
                    # all-masked-row fixup: query 0 attends uniformly
                    for b2 in range(2):
                        nc.vector.memset(p[:, b2 * PB: b2 * PB + 1], 1.0)

                    # AV in 7-window chunks per batch (one PSUM bank each)
                    for b2 in range(2):
                        b = pair * 2 + b2
                        pb = b2 * PB
                        sr = spool.tile([128, 8], F32, tag="sr")
                        for chunk in range(5):
                            w0 = chunk * 7
                            nwin = min(7, W - w0)
                            ot = opool.tile([128, 7 * D], F32, tag="ot")
                            av = avps.tile([128, 512], F32)
                            for k in range(nwin):
                                w = w0 + k
                                own = p[:, pb + w * 256: pb + w * 256 + 128]
                                ov = av[:, k * 65: k * 65 + 65]
                                vw = _ap(vc, w * BL + b, [[W * BL, D + 1]])
                                first_only = w == 0
                                nc.tensor.matmul(ov, own, vw, start=True,
                                                 stop=first_only)
                                if w > 0:
                                    prev = p[:, pb + (w - 1) * 256 + 128:
                                             pb + w * 256]
                                    vprev = _ap(vc, (w - 1) * BL + b,
                                                [[W * BL, D + 1]])
                                    nc.tensor.matmul(ov, prev, vprev,
                                                     start=False, stop=True,
                                                     skip_group_check=True)
                            if chunk == 0:
                                # window-0 query-0 sum correction (+128 pad)
                                nc.vector.tensor_scalar_add(
                                    av[0:1, 64:65], av[0:1, 64:65], 128.0)
                            # normalize: recip of s, broadcast-mul
                            nc.vector.reciprocal(
                                sr[:, :nwin], _ap(av, 64, [[65, nwin]]))
                            avv = _ap(av, 0, [[65, nwin], [1, D]])
                            srv = _ap(sr, 0, [[1, nwin], [0, D]])
                            otv = _ap(ot, 0, [[D, nwin], [1, D]])
                            nc.vector.tensor_mul(otv, avv, srv)
                            dstd = bass.AP(
                                tensor=out, offset=b * N * D + w0 * WS * D,
                                ap=[[D, 128], [WS * D, nwin], [1, D]],
                            )
                            nc.sync.dma_start(
                                out=dstd, in_=_ap(ot, 0, [[D, nwin], [1, D]]))

                emit_attention(0)
                emit_transposes(1, tps2)
                emit_attention(1)

    _split_multiwaits(nc)
    return nc


def _ap_dram(t, offset, dims, row_elems):
    """AP over a [128, row_elems] dram tensor: partition stride row_elems."""
    return bass.AP(tensor=t, offset=offset, ap=[[row_elems, 128]] + dims)


_PROG = None


def _get_prog():
    global _PROG
    if _PROG is None:
        _PROG = _build_program()
    return _PROG


def _host_prep(q, k, v, wq, wk, wv):
    """Build per-core input maps (bf16 casts + layout transforms)."""
    # Toeplitz bands: lo[j,i] = w[d, j-i-1] (prev window), hi[j,i] =
    # w[d, j-i+127] (own window); DRAM layout [j, (d, half, i)] contiguous.
    jj = np.arange(WS)[:, None]
    ii = np.arange(WS)[None, :]
    lod = jj - ii - 1
    hid = jj - ii + (WS - 1)
    lom = (lod >= 0) & (lod < WS)
    him = (hid >= 0) & (hid < WS)
    lodc = np.clip(lod, 0, WS - 1)
    hidc = np.clip(hid, 0, WS - 1)

    def toep(w, scale=1.0):
        wd = np.asarray(w, np.float32).reshape(D, WS) * scale
        t = np.zeros((D, 2, WS, WS), np.float32)
        t[:, 0][:, lom] = wd[:, lodc[lom]]
        t[:, 1][:, him] = wd[:, hidc[him]]
        # [d, half, j, i] -> [j, (d, half, i)]
        return np.ascontiguousarray(t.transpose(2, 0, 1, 3)).reshape(
            WS, D * 2 * WS).astype(NPBF)

    tq_np = toep(wq, SCALE)
    tk_np = toep(wk)
    tv_np = toep(wv)

    theta = 1.0 / ROPE_BASE ** (np.arange(0, D, 2, dtype=np.float32) / D)
    pm = np.arange(N, dtype=np.float32)[:, None] * theta[None, :]
    cos = np.repeat(np.cos(pm), 2, axis=-1)  # [n, d]
    sin = np.repeat(np.sin(pm), 2, axis=-1)
    sgn = np.where(np.arange(D) % 2 == 0, -1.0, 1.0).astype(np.float32)
    # [i, (d, w, b2)] layout, replicated over the 2 batches of a pair
    def rope_table(t):
        tt = np.ascontiguousarray(t.reshape(W, WS, D).transpose(1, 2, 0))
        tt = np.repeat(tt.reshape(WS, D * W, 1), 2, axis=2)
        return np.ascontiguousarray(tt).reshape(WS, D * W * 2).astype(NPBF)

    cosb_np = rope_table(cos)
    sinb_np = rope_table(sin * sgn[None, :])

    def xprep(x, sl):
        # [BL, N, D] -> [j, (d, w, b)] contiguous bf16
        xb = np.asarray(x[sl], np.float32).reshape(BL, W, WS, D)
        return np.ascontiguousarray(xb.transpose(2, 3, 1, 0)).reshape(
            WS, D * W * BL).astype(NPBF)

    in_maps = []
    for c in range(NCORES):
        sl = slice(c * BL, (c + 1) * BL)
        in_maps.append({
            "xq": xprep(q, sl),
            "xk": xprep(k, sl),
            "xv": xprep(v, sl),
            "tq": tq_np, "tk": tk_np, "tv": tv_np,
            "cosb": cosb_np, "sinb": sinb_np,
        })
    return in_maps


def _install_ntff_hook():
    """Provide antenv.axon_hooks with a ctypes NTFF profile hook (the slim
    container lacks it); enables trace=True under axon."""
    import sys as _sys
    import types
    import ctypes
    import contextlib

    try:
        from antenv.axon_hooks import get_axon_ntff_profile_hook  # noqa: F401
        return
    except ImportError:
        pass
    so_path = "/opt/axon/libaxon_pjrt.so"
    try:
        lib = ctypes.CDLL(so_path)
    except OSError:
        return
    if not hasattr(lib, "axon_start_nrt_profile"):
        return
    lib.axon_start_nrt_profile.argtypes = [
        ctypes.POINTER(ctypes.c_int64), ctypes.c_size_t]
    lib.axon_start_nrt_profile.restype = ctypes.c_int64
    lib.axon_stop_nrt_profile.argtypes = [ctypes.c_char_p]
    lib.axon_stop_nrt_profile.restype = ctypes.c_int64

    @contextlib.contextmanager
    def _hook(output_dir, device_ids):
        import jax
        jax.devices()
        if device_ids:
            ids = (ctypes.c_int64 * len(device_ids))(*device_ids)
            rc = lib.axon_start_nrt_profile(ids, len(device_ids))
        else:
            rc = lib.axon_start_nrt_profile(None, 0)
        if rc != 0:
            raise RuntimeError(f"axon_start_nrt_profile rc={rc}")
        try:
            yield
        finally:
            n = lib.axon_stop_nrt_profile(str(output_dir).encode())
            print(f"profile: {n} file(s) written to {output_dir}")

    import antenv

    mod = types.ModuleType("antenv.axon_hooks")
    _state = {"hook": _hook}
    mod.set_axon_ntff_profile_hook = lambda h: _state.__setitem__("hook", h)
    mod.get_axon_ntff_profile_hook = lambda: _state["hook"]
    _sys.modules["antenv.axon_hooks"] = mod
    antenv.axon_hooks = mod


def run(q, k, v, wq, wk, wv, trace=False):
    nc = _get_prog()
    in_maps = _host_prep(q, k, v, wq, wk, wv)
    if trace:
        _install_ntff_hook()
    res = run_bass_kernel_spmd(nc, in_maps, core_ids=list(range(NCORES)),
                               trace=trace)
    outp = np.concatenate([res.results[c]["out"] for c in range(NCORES)], axis=0)
    return outp, res


def kernel(q, k, v, wq, wk, wv):
    outp, _ = run(q, k, v, wq, wk, wv)
    return outp
